# revision 12
# baseline (speedup 1.0000x reference)
"""Trainium2 Bass kernel for nn_Diffusion_57818849739555.

Computes one gradient step of pre_rot/trans through a pairwise
atn/dist energy, including the QR-decomposition VJP, returning
(final_rot [32,3,3], final_trans [32,3]) as float32.

Strategy (8 NeuronCores, data-parallel over the B=8 graph dim):
  Host (fp64, tiny tensors): coordinate centering, QR of pre_rot,
    rotated ligand coords y, bf16 triple-splits for the augmented
    distance matmul, fp16 split of rc for the reduction weights.
  Device (per core, one graph): the heavy [T*L=2048, R=1536] work —
    - atn^T = rec_feat @ lig_feat^T                    (PE, fp32)
    - d2    = |y|^2 + |rc|^2 - 2 y.rc  via a K=24 bf16 triple-split
              augmented matmul (error ~1e-7, full fp32-grade d2)   (PE)
    - p     = SC * d2^{-3/2} = Exp(-1.5 * Ln(d2) + ln SC)          (ACT)
    - w     = p * atn  (broadcast over T)  -> fp16                 (DVE)
    - [V|W] = [rc_h|rc_l|1]^T @ w  accumulated over r-tiles        (PE)
  Host (fp64): g_y from [V|W], gradient w.r.t. Q/trans, manual QR
    VJP, final QR + outputs.
"""

import math

import numpy as np
import ml_dtypes

import bass_rust
import concourse.bass as bass
import concourse.tile as tile
from concourse import mybir
from concourse.bass_utils import run_bass_kernel_spmd
from concourse.tile import TileContext
from concourse.vector_clock import ScopedClock

BF16 = ml_dtypes.bfloat16

B, L, R, F, T = 8, 64, 1536, 64, 32
K = T * L           # 2048 flattened (t, l)
RT = R // 128       # 12 r-tiles
SC = 2.0 ** -4      # fp16 overflow guard for w; divided back out on host
GRAD_COEF = 1.0

# Close pairs (d2 < D0) are clamped on device and corrected exactly on the
# host: the PE's sequential fp32 accumulation of the augmented matmul leaves
# ~1e-4..1e-3 absolute noise on d2, which d2^{-3/2} amplifies catastrophically
# for the gradient-dominating close pairs.  Clamping bounds the relative
# error at 1.5*eps/D0; the host recomputes the clamped pairs in fp64.
D0 = 9.0
T_MAX = SC * D0 ** -1.5

# Results of the last device run (for test harnesses).
LAST_RESULTS = None
LAST_IN_MAPS = None

# ----------------------------------------------------------------------------
# TileContext tail-drain fix: this walrus build rejects >1 sync wait on the
# final CTRL Drain instruction ("Too many sync wait commands").  Distribute
# the waits onto single-wait sync NOPs issued just before the drain (the SP
# engine executes them in order, so the drain still observes all of them).
# ----------------------------------------------------------------------------
def _patched_drain_and_barrier(self, tick_clock, wait_clock):
    probe = self.nc.sync.nop()
    wait_clock.add_sem_waits(
        probe.ins, ScopedClock({None: tick_clock.global_clock})
    )
    si = probe.ins.sync_info
    waits = list(si.on_wait) if si and si.on_wait else []
    if len(waits) > 1:
        probe.ins.sync_info = bass_rust.SyncInfo(
            on_wait=waits[:1], on_update=list(si.on_update or [])
        )
        for w in waits[1:]:
            extra = self.nc.sync.nop()
            extra.ins.sync_info = bass_rust.SyncInfo(on_wait=[w], on_update=[])
    self.nc.sync.drain()
    self.nc.all_engine_barrier()
    assert self.sems is not None
    popped = self.nc._tile_sem_poison_stack.pop()
    assert popped is self._sem_poison
    self.nc.clear_and_free_semaphores(list(self.sems.allocated().values()))
    self.nc.all_engine_barrier()


def _install_drain_patch():
    if getattr(TileContext, "_ant_drain_patch", False):
        return
    TileContext._drain_and_barrier = _patched_drain_and_barrier
    TileContext._ant_drain_patch = True


_SPLIT_SEQ = [0]


def _split_multi_waits(nc):
    """Walrus here rejects >1 sync wait on an instruction: hoist extra waits
    onto same-engine NOPs inserted immediately before the instruction."""
    for fn in nc.m.functions:
        for bb in fn.blocks:
            insts = bb.instructions
            out = []
            for inst in insts:
                si = inst.sync_info
                waits = list(si.on_wait) if si and si.on_wait else []
                if len(waits) > 1 and inst.engine != mybir.EngineType.Unassigned:
                    for w in waits[:-1]:
                        _SPLIT_SEQ[0] += 1
                        nop = mybir.InstNoOp(
                            name=f"antwaitsplit-{_SPLIT_SEQ[0]}", ins=[], outs=[]
                        )
                        nop.engine = inst.engine
                        nop.sync_info = bass_rust.SyncInfo(
                            on_wait=[w], on_update=[]
                        )
                        out.append(nop)
                    inst.sync_info = bass_rust.SyncInfo(
                        on_wait=waits[-1:], on_update=list(si.on_update or [])
                    )
                out.append(inst)
            bb.instructions = out


# ----------------------------------------------------------------------------
# Device program
# ----------------------------------------------------------------------------
_CACHED_NC = None


def _build_device_program():
    global _CACHED_NC
    if _CACHED_NC is not None:
        return _CACHED_NC
    _install_drain_patch()

    f32, f16, bf16 = mybir.dt.float32, mybir.dt.float16, mybir.dt.bfloat16
    ACT = mybir.ActivationFunctionType

    nc = bass.Bass()
    d_lhsT = nc.declare_dram_parameter("aug_lhsT", [24, R], bf16, isOutput=False)
    d_rhs = nc.declare_dram_parameter("aug_rhs", [24, K], bf16, isOutput=False)
    d_rc17 = nc.declare_dram_parameter("rc17", [RT, 128, 7], f16, isOutput=False)
    d_recT = nc.declare_dram_parameter("recT", [F, R], f32, isOutput=False)
    d_ligT = nc.declare_dram_parameter("ligT", [F, L], f32, isOutput=False)
    d_vw = nc.declare_dram_parameter("vw", [7, K], f32, isOutput=True)

    with TileContext(nc) as tc:
        with tc.tile_pool(name="consts", bufs=1) as consts:
            s_lhsT = consts.tile([24, R], bf16)
            nc.sync.dma_start(out=s_lhsT, in_=d_lhsT[:, :])
            s_rhs = consts.tile([24, K], bf16)
            nc.sync.dma_start(out=s_rhs, in_=d_rhs[:, :])
            s_rc17 = consts.tile([128, RT, 7], f16)
            nc.sync.dma_start(
                out=s_rc17, in_=d_rc17[:, :, :].rearrange("r p c -> p r c")
            )
            s_ligT = consts.tile([F, L], f32)
            nc.sync.dma_start(out=s_ligT, in_=d_ligT[:, :])
            s_recT = consts.tile([F, R], f32)
            nc.sync.dma_start(out=s_recT, in_=d_recT[:, :])
            s_atn = consts.tile([128, RT, L], f32)

            # Phase 1: atn^T[r, l] = sum_f rec[r, f] * lig[l, f]
            with tc.tile_pool(name="atn_ps", bufs=2, space="PSUM") as atn_ps:
                for rt in range(RT):
                    ap = atn_ps.tile([128, L], f32)
                    nc.tensor.matmul(
                        ap,
                        lhsT=s_recT[:, rt * 128 : (rt + 1) * 128],
                        rhs=s_ligT,
                        start=True,
                        stop=True,
                    )
                    nc.vector.tensor_copy(s_atn[:, rt, :], ap)

            # Phase 2: main pipeline over the 12 r-tiles
            with (
                tc.tile_pool(name="u", bufs=2) as u_pool,
                tc.tile_pool(name="t", bufs=2) as t_pool,
                tc.tile_pool(name="w", bufs=2) as w_pool,
                tc.tile_pool(name="d2", bufs=2, space="PSUM") as d2_pool,
                tc.tile_pool(name="vw", bufs=1, space="PSUM") as vw_pool,
            ):
                vw_ps = vw_pool.tile([7, K], f32)
                for rt in range(RT):
                    u_t = u_pool.tile([128, K], f32)
                    t_t = t_pool.tile([128, K], f32)
                    w_t = w_pool.tile([128, K], f16)
                    lhsT_rt = s_lhsT[:, rt * 128 : (rt + 1) * 128]
                    for h in range(2):  # 1024-column halves (2 PSUM banks)
                        d2p = d2_pool.tile([128, 1024], f32)
                        for q in range(2):  # 512-column matmul chunks
                            c0 = h * 1024 + q * 512
                            nc.tensor.matmul(
                                d2p[:, q * 512 : (q + 1) * 512],
                                lhsT=lhsT_rt,
                                rhs=s_rhs[:, c0 : c0 + 512],
                                start=True,
                                stop=True,
                            )
                        # Ln(SC^(-2/3) * d2) = ln d2 - (2/3) ln SC, so the
                        # Exp(-1.5 * u) below yields SC * d2^{-3/2} exactly.
                        nc.scalar.activation(
                            out=u_t[:, h * 1024 : (h + 1) * 1024],
                            in_=d2p,
                            func=ACT.Ln,
                            scale=float(SC ** (-2.0 / 3.0)),
                        )
                    # p = SC * d2^{-3/2}
                    nc.scalar.activation(
                        out=t_t,
                        in_=u_t,
                        func=ACT.Exp,
                        scale=-1.5,
                    )
                    # w = min(p, T_MAX) * atn (atn broadcast along the T dim)
                    atn_b = s_atn[:, rt, :].unsqueeze(1).broadcast_to([128, T, L])
                    nc.vector.scalar_tensor_tensor(
                        out=w_t[:, :].rearrange("p (t l) -> p t l", t=T),
                        in0=t_t[:, :].rearrange("p (t l) -> p t l", t=T),
                        scalar=float(T_MAX),
                        in1=atn_b,
                        op0=mybir.AluOpType.min,
                        op1=mybir.AluOpType.mult,
                    )
                    # [V|W] accumulation over r-tiles
                    for c in range(4):
                        nc.tensor.matmul(
                            vw_ps[:, c * 512 : (c + 1) * 512],
                            lhsT=s_rc17[:, rt, :],
                            rhs=w_t[:, c * 512 : (c + 1) * 512],
                            start=(rt == 0),
                            stop=(rt == RT - 1),
                        )
                s_out = consts.tile([7, K], f32)
                nc.vector.tensor_copy(s_out, vw_ps)
                nc.sync.dma_start(out=d_vw[:, :], in_=s_out)

    _split_multi_waits(nc)
    _CACHED_NC = nc
    return nc


# ----------------------------------------------------------------------------
# Host-side math (fp64)
# ----------------------------------------------------------------------------
def _split3_bf16(x):
    h = x.astype(BF16).astype(np.float64)
    m = (x - h).astype(BF16).astype(np.float64)
    l = (x - h - m).astype(BF16).astype(np.float64)
    return h, m, l


def _qr_q(A):
    return np.stack([np.linalg.qr(A[t])[0] for t in range(A.shape[0])])


def _qr_vjp(A, gQ):
    """VJP of A -> qr(A).Q for square invertible A (batched [T,3,3])."""
    out = np.zeros_like(A)
    for t in range(A.shape[0]):
        Q, Rm = np.linalg.qr(A[t])
        M = Q.T @ gQ[t]
        P = np.tril(M - M.T, -1)
        out[t] = Q @ P @ np.linalg.inv(Rm).T
    return out


def _prepare_core_inputs(lig_feat, rec_feat, lig_coord, rec_coord, Q, trans):
    """Build the per-core (per-graph) input map. All args fp64 except feats."""
    lc = lig_coord - lig_coord.mean(0, keepdims=True)      # [L,3]
    rc = rec_coord - rec_coord.mean(0, keepdims=True)      # [R,3]
    y = np.einsum("tij,lj->tli", Q, lc) + trans[:, None, :]  # [T,L,3]
    yf = y.reshape(K, 3)
    a = -2.0 * yf                                           # [K,3]
    ny2 = (yf ** 2).sum(-1)                                 # [K]
    nr2 = (rc ** 2).sum(-1)                                 # [R]

    ah, am, al = _split3_bf16(a)
    bh, bm, bl = _split3_bf16(rc)
    nyh, nym, nyl = _split3_bf16(ny2)
    nrh, nrm, nrl = _split3_bf16(nr2)

    ones_r = np.ones(R)
    ones_k = np.ones(K)
    # Row order matters: the PE accumulates rows sequentially in fp32, so put
    # the big mutually-cancelling terms first (norms then the h*h cross terms)
    # to keep the running partial sums small for the remaining rows.
    lhs_rows, rhs_rows = [], []
    lhs_rows.append(nrh); rhs_rows.append(ones_k)
    lhs_rows.append(ones_r); rhs_rows.append(nyh)
    for i in range(3):
        lhs_rows.append(bh[:, i]); rhs_rows.append(ah[:, i])
    lhs_rows.append(nrm); rhs_rows.append(ones_k)
    lhs_rows.append(ones_r); rhs_rows.append(nym)
    for i in range(3):
        lhs_rows.append(bh[:, i]); rhs_rows.append(am[:, i])
        lhs_rows.append(bm[:, i]); rhs_rows.append(ah[:, i])
    lhs_rows.append(nrl); rhs_rows.append(ones_k)
    lhs_rows.append(ones_r); rhs_rows.append(nyl)
    for i in range(3):
        lhs_rows.append(bh[:, i]); rhs_rows.append(al[:, i])
        lhs_rows.append(bl[:, i]); rhs_rows.append(ah[:, i])
        lhs_rows.append(bm[:, i]); rhs_rows.append(am[:, i])

    aug_lhsT = np.stack(lhs_rows).astype(BF16)              # [24, R]
    aug_rhs = np.stack(rhs_rows).astype(BF16)               # [24, K]

    rc_h = rc.astype(np.float16).astype(np.float64)
    rc_l = (rc - rc_h).astype(np.float16).astype(np.float64)
    rc17 = np.empty((R, 7))
    rc17[:, 0:3] = rc_h
    rc17[:, 3:6] = rc_l
    rc17[:, 6] = 1.0
    rc17 = rc17.reshape(RT, 128, 7).astype(np.float16)

    return {
        "aug_lhsT": np.ascontiguousarray(aug_lhsT),
        "aug_rhs": np.ascontiguousarray(aug_rhs),
        "rc17": np.ascontiguousarray(rc17),
        "recT": np.ascontiguousarray(rec_feat.T.astype(np.float32)),
        "ligT": np.ascontiguousarray(lig_feat.T.astype(np.float32)),
    }, lc, rc, y


def _close_pair_corrections(lig_feat, rec_feat, y, rc):
    """Exact fp64 (W, V) contributions of pairs with d2 < D0, replacing the
    device's clamped value D0^{-3/2}. Returns (Wc [T,L], Vc [T,L,3])."""
    from scipy.spatial import cKDTree

    yf = y.reshape(K, 3)
    tree = cKDTree(rc)
    Wc = np.zeros(K)
    Vc = np.zeros((K, 3))
    neigh = tree.query_ball_point(yf, np.sqrt(D0))
    ks, rs = [], []
    for k, lst in enumerate(neigh):
        for r in lst:
            ks.append(k)
            rs.append(r)
    if ks:
        ks = np.asarray(ks)
        rs = np.asarray(rs)
        d2e = ((yf[ks] - rc[rs]) ** 2).sum(-1)
        keep = d2e < D0
        ks, rs, d2e = ks[keep], rs[keep], d2e[keep]
        ls = ks % L
        atn = (lig_feat[ls] * rec_feat[rs]).sum(-1)
        dp = atn * (d2e ** -1.5 - D0 ** -1.5)
        np.add.at(Wc, ks, dp)
        np.add.at(Vc, ks, dp[:, None] * rc[rs])
    return Wc.reshape(T, L), Vc.reshape(T, L, 3)


def kernel(lig_feat, rec_feat, lig_coord, rec_coord, pre_rot, trans):
    global LAST_RESULTS
    lig_feat = np.asarray(lig_feat)
    rec_feat = np.asarray(rec_feat)
    lig_coord = np.asarray(lig_coord, dtype=np.float64)
    rec_coord = np.asarray(rec_coord, dtype=np.float64)
    pre_rot64 = np.asarray(pre_rot, dtype=np.float64)
    trans64 = np.asarray(trans, dtype=np.float64)

    Q = _qr_q(pre_rot64)                                    # [T,3,3]

    in_maps, lcs, rcs, ys = [], [], [], []
    for b in range(B):
        m, lc, rc, y = _prepare_core_inputs(
            lig_feat[b], rec_feat[b], lig_coord[b], rec_coord[b], Q, trans64
        )
        in_maps.append(m)
        lcs.append(lc)
        rcs.append(rc)
        ys.append(y)

    global LAST_IN_MAPS
    LAST_IN_MAPS = in_maps
    nc = _build_device_program()
    res = run_bass_kernel_spmd(nc, in_maps, list(range(B)))
    LAST_RESULTS = res

    c0 = -1.0 / (B * T)
    g_trans = np.zeros((T, 3))
    g_Q = np.zeros((T, 3, 3))
    for b in range(B):
        vw = np.asarray(res.results[b]["vw"], dtype=np.float64)  # [7, K]
        V = (vw[0:3] + vw[3:6]).T.reshape(T, L, 3) / SC
        W = vw[6].reshape(T, L) / SC
        # exact fp64 correction of the clamped close pairs
        Wc, Vc = _close_pair_corrections(
            lig_feat[b].astype(np.float64),
            rec_feat[b].astype(np.float64),
            ys[b],
            rcs[b],
        )
        W = W + Wc
        V = V + Vc
        gy = c0 * (W[..., None] * ys[b] - V)                 # [T,L,3]
        g_trans += gy.sum(1)
        g_Q += np.einsum("tli,lj->tij", gy, lcs[b])

    g_pre = _qr_vjp(pre_rot64, g_Q)
    final_rot = _qr_q(pre_rot64 - GRAD_COEF * g_pre)
    final_trans = trans64 - GRAD_COEF * g_trans
    return final_rot.astype(np.float32), final_trans.astype(np.float32)


# revision 16
# speedup vs baseline: 6126.6497x; 6126.6497x over previous
"""Trainium2 Bass kernel for nn_Diffusion_57818849739555.

Computes one gradient step of pre_rot/trans through a pairwise
atn/dist energy, including the QR-decomposition VJP, returning
(final_rot [32,3,3], final_trans [32,3]) as float32.

Strategy (8 NeuronCores, data-parallel over the B=8 graph dim):
  Host (fp64, tiny tensors): coordinate centering, QR of pre_rot,
    rotated ligand coords y, bf16 triple-splits for the augmented
    distance matmul, fp16 split of rc for the reduction weights.
  Device (per core, one graph): the heavy [T*L=2048, R=1536] work —
    - atn^T = rec_feat @ lig_feat^T                    (PE, fp32)
    - d2    = |y|^2 + |rc|^2 - 2 y.rc  via a K=24 bf16 triple-split
              augmented matmul (error ~1e-7, full fp32-grade d2)   (PE)
    - p     = SC * d2^{-3/2} = Exp(-1.5 * Ln(d2) + ln SC)          (ACT)
    - w     = p * atn  (broadcast over T)  -> fp16                 (DVE)
    - [V|W] = [rc_h|rc_l|1]^T @ w  accumulated over r-tiles        (PE)
  Host (fp64): g_y from [V|W], gradient w.r.t. Q/trans, manual QR
    VJP, final QR + outputs.
"""

import math

import numpy as np
import ml_dtypes

import bass_rust
import concourse.bass as bass
import concourse.tile as tile
from concourse import mybir
from concourse.bass_utils import run_bass_kernel_spmd
from concourse.tile import TileContext
from concourse.vector_clock import ScopedClock

BF16 = ml_dtypes.bfloat16

B, L, R, F, T = 8, 64, 1536, 64, 32
K = T * L           # 2048 flattened (t, l)
RT = R // 128       # 12 r-tiles
SC = 2.0 ** -4      # fp16 overflow guard for w; divided back out on host
GRAD_COEF = 1.0

# Close pairs (d2 < D0) are clamped on device and corrected exactly on the
# host: the PE's sequential fp32 accumulation of the augmented matmul leaves
# ~1e-4..1e-3 absolute noise on d2, which d2^{-3/2} amplifies catastrophically
# for the gradient-dominating close pairs.  Clamping bounds the relative
# error at 1.5*eps/D0; the host recomputes the clamped pairs in fp64.
D0 = 9.0
T_MAX = SC * D0 ** -1.5

# Results of the last device run (for test harnesses).
LAST_RESULTS = None
LAST_IN_MAPS = None

# ----------------------------------------------------------------------------
# TileContext tail-drain fix: this walrus build rejects >1 sync wait on the
# final CTRL Drain instruction ("Too many sync wait commands").  Distribute
# the waits onto single-wait sync NOPs issued just before the drain (the SP
# engine executes them in order, so the drain still observes all of them).
# ----------------------------------------------------------------------------
def _patched_drain_and_barrier(self, tick_clock, wait_clock):
    probe = self.nc.sync.nop()
    wait_clock.add_sem_waits(
        probe.ins, ScopedClock({None: tick_clock.global_clock})
    )
    si = probe.ins.sync_info
    waits = list(si.on_wait) if si and si.on_wait else []
    if len(waits) > 1:
        probe.ins.sync_info = bass_rust.SyncInfo(
            on_wait=waits[:1], on_update=list(si.on_update or [])
        )
        for w in waits[1:]:
            extra = self.nc.sync.nop()
            extra.ins.sync_info = bass_rust.SyncInfo(on_wait=[w], on_update=[])
    self.nc.sync.drain()
    self.nc.all_engine_barrier()
    assert self.sems is not None
    popped = self.nc._tile_sem_poison_stack.pop()
    assert popped is self._sem_poison
    self.nc.clear_and_free_semaphores(list(self.sems.allocated().values()))
    self.nc.all_engine_barrier()


def _install_drain_patch():
    if getattr(TileContext, "_ant_drain_patch", False):
        return
    TileContext._drain_and_barrier = _patched_drain_and_barrier
    TileContext._ant_drain_patch = True


_SPLIT_SEQ = [0]


def _split_multi_waits(nc):
    """Walrus here rejects >1 sync wait on an instruction: hoist extra waits
    onto same-engine NOPs inserted immediately before the instruction."""
    for fn in nc.m.functions:
        for bb in fn.blocks:
            insts = bb.instructions
            out = []
            for inst in insts:
                si = inst.sync_info
                waits = list(si.on_wait) if si and si.on_wait else []
                if len(waits) > 1 and inst.engine != mybir.EngineType.Unassigned:
                    for w in waits[:-1]:
                        _SPLIT_SEQ[0] += 1
                        nop = mybir.InstNoOp(
                            name=f"antwaitsplit-{_SPLIT_SEQ[0]}", ins=[], outs=[]
                        )
                        nop.engine = inst.engine
                        nop.sync_info = bass_rust.SyncInfo(
                            on_wait=[w], on_update=[]
                        )
                        out.append(nop)
                    inst.sync_info = bass_rust.SyncInfo(
                        on_wait=waits[-1:], on_update=list(si.on_update or [])
                    )
                out.append(inst)
            bb.instructions = out


# ----------------------------------------------------------------------------
# Device program
# ----------------------------------------------------------------------------
_CACHED_NC = {}


def _build_device_program(reps=1):
    """reps>1 repeats the whole compute body (timing experiments only)."""
    if reps in _CACHED_NC:
        return _CACHED_NC[reps]
    _install_drain_patch()

    f32, f16, bf16 = mybir.dt.float32, mybir.dt.float16, mybir.dt.bfloat16
    ACT = mybir.ActivationFunctionType

    nc = bass.Bass()
    d_lhsT = nc.declare_dram_parameter("aug_lhsT", [24, R], bf16, isOutput=False)
    d_rhs = nc.declare_dram_parameter("aug_rhs", [24, K], bf16, isOutput=False)
    d_rc17 = nc.declare_dram_parameter("rc17", [RT, 128, 7], f16, isOutput=False)
    d_recT = nc.declare_dram_parameter("recT", [F, R], f32, isOutput=False)
    d_ligT = nc.declare_dram_parameter("ligT", [F, L], f32, isOutput=False)
    d_vw = nc.declare_dram_parameter("vw", [7, K], f32, isOutput=True)

    with TileContext(nc) as tc:
        with tc.tile_pool(name="consts", bufs=1) as consts:
            # DMA order matters: the d2 pipeline (the critical path) only
            # needs aug_lhsT + aug_rhs; the big recT load comes last.
            s_lhsT = consts.tile([24, R], bf16)
            nc.sync.dma_start(out=s_lhsT, in_=d_lhsT[:, :])
            s_rhs = consts.tile([24, K], bf16)
            nc.sync.dma_start(out=s_rhs, in_=d_rhs[:, :])
            s_rc17 = consts.tile([128, RT, 7], f16)
            nc.sync.dma_start(
                out=s_rc17, in_=d_rc17[:, :, :].rearrange("r p c -> p r c")
            )
            s_ligT = consts.tile([F, L], f32)
            nc.sync.dma_start(out=s_ligT, in_=d_ligT[:, :])
            s_recT = consts.tile([F, R], f32)
            nc.sync.dma_start(out=s_recT, in_=d_recT[:, :])
            s_atn = consts.tile([128, RT, L], f32)

            with (
                tc.tile_pool(name="u", bufs=2) as u_pool,
                tc.tile_pool(name="t", bufs=2) as t_pool,
                tc.tile_pool(name="w", bufs=2) as w_pool,
                tc.tile_pool(name="d2", bufs=2, space="PSUM") as d2_pool,
            ):
                # atn^T[r, l] = sum_f rec[r, f] * lig[l, f]; single-buffered
                # PSUM bank, freed before the vw accumulator is allocated.
                # Runs concurrently with the early d2 pipeline below.
                with tc.tile_pool(name="atn_ps", bufs=1, space="PSUM") as atn_ps:
                    for rt in range(RT):
                        ap = atn_ps.tile([128, L], f32)
                        nc.tensor.matmul(
                            ap,
                            lhsT=s_recT[:, rt * 128 : (rt + 1) * 128],
                            rhs=s_ligT,
                            start=True,
                            stop=True,
                        )
                        nc.vector.tensor_copy(s_atn[:, rt, :], ap)

                with tc.tile_pool(name="vw", bufs=1, space="PSUM") as vw_pool:
                    vw_ps = vw_pool.tile([7, K], f32)
                    s_out = consts.tile([7, K], f32)
                    for rt in range(RT * reps):
                        rt = rt % RT
                        u_t = u_pool.tile([128, K], f32)
                        t_t = t_pool.tile([128, K], f32)
                        w_t = w_pool.tile([128, K], f16)
                        lhsT_rt = s_lhsT[:, rt * 128 : (rt + 1) * 128]
                        for h in range(2):  # 1024-column halves (2 PSUM banks)
                            d2p = d2_pool.tile([128, 1024], f32)
                            for q in range(2):  # 512-column matmul chunks
                                c0 = h * 1024 + q * 512
                                nc.tensor.matmul(
                                    d2p[:, q * 512 : (q + 1) * 512],
                                    lhsT=lhsT_rt,
                                    rhs=s_rhs[:, c0 : c0 + 512],
                                    start=True,
                                    stop=True,
                                )
                            # Ln(SC^(-2/3) * d2) = ln d2 - (2/3) ln SC, so
                            # Exp(-1.5 * u) below yields SC * d2^{-3/2}.
                            nc.scalar.activation(
                                out=u_t[:, h * 1024 : (h + 1) * 1024],
                                in_=d2p,
                                func=ACT.Ln,
                                scale=float(SC ** (-2.0 / 3.0)),
                            )
                        # p = SC * d2^{-3/2}
                        nc.scalar.activation(
                            out=t_t,
                            in_=u_t,
                            func=ACT.Exp,
                            scale=-1.5,
                        )
                        # w = min(p, T_MAX) * atn, chunked so the final
                        # tile's [V|W] columns complete (and drain) early.
                        atn_b = (
                            s_atn[:, rt, :]
                            .unsqueeze(1)
                            .broadcast_to([128, T // 4, L])
                        )
                        for c in range(4):
                            cs = slice(c * 512, (c + 1) * 512)
                            nc.vector.scalar_tensor_tensor(
                                out=w_t[:, cs].rearrange(
                                    "p (t l) -> p t l", l=L
                                ),
                                in0=t_t[:, cs].rearrange(
                                    "p (t l) -> p t l", l=L
                                ),
                                scalar=float(T_MAX),
                                in1=atn_b,
                                op0=mybir.AluOpType.min,
                                op1=mybir.AluOpType.mult,
                            )
                            nc.tensor.matmul(
                                vw_ps[:, cs],
                                lhsT=s_rc17[:, rt, :],
                                rhs=w_t[:, cs],
                                start=(rt == 0),
                                stop=(rt == RT - 1),
                            )
                            if rt == RT - 1:
                                nc.vector.tensor_copy(s_out[:, cs], vw_ps[:, cs])
                                nc.sync.dma_start(
                                    out=d_vw[:, cs], in_=s_out[:, cs]
                                )

    _split_multi_waits(nc)
    _CACHED_NC[reps] = nc
    return nc


# ----------------------------------------------------------------------------
# Host-side math (fp64)
# ----------------------------------------------------------------------------
def _split3_bf16(x):
    h = x.astype(BF16).astype(np.float64)
    m = (x - h).astype(BF16).astype(np.float64)
    l = (x - h - m).astype(BF16).astype(np.float64)
    return h, m, l


def _qr_q(A):
    return np.stack([np.linalg.qr(A[t])[0] for t in range(A.shape[0])])


def _qr_vjp(A, gQ):
    """VJP of A -> qr(A).Q for square invertible A (batched [T,3,3])."""
    out = np.zeros_like(A)
    for t in range(A.shape[0]):
        Q, Rm = np.linalg.qr(A[t])
        M = Q.T @ gQ[t]
        P = np.tril(M - M.T, -1)
        out[t] = Q @ P @ np.linalg.inv(Rm).T
    return out


def _prepare_core_inputs(lig_feat, rec_feat, lig_coord, rec_coord, Q, trans):
    """Build the per-core (per-graph) input map. All args fp64 except feats."""
    lc = lig_coord - lig_coord.mean(0, keepdims=True)      # [L,3]
    rc = rec_coord - rec_coord.mean(0, keepdims=True)      # [R,3]
    y = np.einsum("tij,lj->tli", Q, lc) + trans[:, None, :]  # [T,L,3]
    yf = y.reshape(K, 3)
    a = -2.0 * yf                                           # [K,3]
    ny2 = (yf ** 2).sum(-1)                                 # [K]
    nr2 = (rc ** 2).sum(-1)                                 # [R]

    ah, am, al = _split3_bf16(a)
    bh, bm, bl = _split3_bf16(rc)
    nyh, nym, nyl = _split3_bf16(ny2)
    nrh, nrm, nrl = _split3_bf16(nr2)

    ones_r = np.ones(R)
    ones_k = np.ones(K)
    # Row order matters: the PE accumulates rows sequentially in fp32, so put
    # the big mutually-cancelling terms first (norms then the h*h cross terms)
    # to keep the running partial sums small for the remaining rows.
    lhs_rows, rhs_rows = [], []
    lhs_rows.append(nrh); rhs_rows.append(ones_k)
    lhs_rows.append(ones_r); rhs_rows.append(nyh)
    for i in range(3):
        lhs_rows.append(bh[:, i]); rhs_rows.append(ah[:, i])
    lhs_rows.append(nrm); rhs_rows.append(ones_k)
    lhs_rows.append(ones_r); rhs_rows.append(nym)
    for i in range(3):
        lhs_rows.append(bh[:, i]); rhs_rows.append(am[:, i])
        lhs_rows.append(bm[:, i]); rhs_rows.append(ah[:, i])
    lhs_rows.append(nrl); rhs_rows.append(ones_k)
    lhs_rows.append(ones_r); rhs_rows.append(nyl)
    for i in range(3):
        lhs_rows.append(bh[:, i]); rhs_rows.append(al[:, i])
        lhs_rows.append(bl[:, i]); rhs_rows.append(ah[:, i])
        lhs_rows.append(bm[:, i]); rhs_rows.append(am[:, i])

    aug_lhsT = np.stack(lhs_rows).astype(BF16)              # [24, R]
    aug_rhs = np.stack(rhs_rows).astype(BF16)               # [24, K]

    rc_h = rc.astype(np.float16).astype(np.float64)
    rc_l = (rc - rc_h).astype(np.float16).astype(np.float64)
    rc17 = np.empty((R, 7))
    rc17[:, 0:3] = rc_h
    rc17[:, 3:6] = rc_l
    rc17[:, 6] = 1.0
    rc17 = rc17.reshape(RT, 128, 7).astype(np.float16)

    return {
        "aug_lhsT": np.ascontiguousarray(aug_lhsT),
        "aug_rhs": np.ascontiguousarray(aug_rhs),
        "rc17": np.ascontiguousarray(rc17),
        "recT": np.ascontiguousarray(rec_feat.T.astype(np.float32)),
        "ligT": np.ascontiguousarray(lig_feat.T.astype(np.float32)),
    }, lc, rc, y


def _close_pair_corrections(lig_feat, rec_feat, y, rc):
    """Exact fp64 (W, V) contributions of pairs with d2 < D0, replacing the
    device's clamped value D0^{-3/2}. Returns (Wc [T,L], Vc [T,L,3])."""
    from scipy.spatial import cKDTree

    yf = y.reshape(K, 3)
    tree = cKDTree(rc)
    Wc = np.zeros(K)
    Vc = np.zeros((K, 3))
    neigh = tree.query_ball_point(yf, np.sqrt(D0))
    ks, rs = [], []
    for k, lst in enumerate(neigh):
        for r in lst:
            ks.append(k)
            rs.append(r)
    if ks:
        ks = np.asarray(ks)
        rs = np.asarray(rs)
        d2e = ((yf[ks] - rc[rs]) ** 2).sum(-1)
        keep = d2e < D0
        ks, rs, d2e = ks[keep], rs[keep], d2e[keep]
        ls = ks % L
        atn = (lig_feat[ls] * rec_feat[rs]).sum(-1)
        dp = atn * (d2e ** -1.5 - D0 ** -1.5)
        np.add.at(Wc, ks, dp)
        np.add.at(Vc, ks, dp[:, None] * rc[rs])
    return Wc.reshape(T, L), Vc.reshape(T, L, 3)


def kernel(lig_feat, rec_feat, lig_coord, rec_coord, pre_rot, trans):
    global LAST_RESULTS
    lig_feat = np.asarray(lig_feat)
    rec_feat = np.asarray(rec_feat)
    lig_coord = np.asarray(lig_coord, dtype=np.float64)
    rec_coord = np.asarray(rec_coord, dtype=np.float64)
    pre_rot64 = np.asarray(pre_rot, dtype=np.float64)
    trans64 = np.asarray(trans, dtype=np.float64)

    Q = _qr_q(pre_rot64)                                    # [T,3,3]

    in_maps, lcs, rcs, ys = [], [], [], []
    for b in range(B):
        m, lc, rc, y = _prepare_core_inputs(
            lig_feat[b], rec_feat[b], lig_coord[b], rec_coord[b], Q, trans64
        )
        in_maps.append(m)
        lcs.append(lc)
        rcs.append(rc)
        ys.append(y)

    global LAST_IN_MAPS
    LAST_IN_MAPS = in_maps
    nc = _build_device_program()
    res = run_bass_kernel_spmd(nc, in_maps, list(range(B)))
    LAST_RESULTS = res

    c0 = -1.0 / (B * T)
    g_trans = np.zeros((T, 3))
    g_Q = np.zeros((T, 3, 3))
    for b in range(B):
        vw = np.asarray(res.results[b]["vw"], dtype=np.float64)  # [7, K]
        V = (vw[0:3] + vw[3:6]).T.reshape(T, L, 3) / SC
        W = vw[6].reshape(T, L) / SC
        # exact fp64 correction of the clamped close pairs
        Wc, Vc = _close_pair_corrections(
            lig_feat[b].astype(np.float64),
            rec_feat[b].astype(np.float64),
            ys[b],
            rcs[b],
        )
        W = W + Wc
        V = V + Vc
        gy = c0 * (W[..., None] * ys[b] - V)                 # [T,L,3]
        g_trans += gy.sum(1)
        g_Q += np.einsum("tli,lj->tij", gy, lcs[b])

    g_pre = _qr_vjp(pre_rot64, g_Q)
    final_rot = _qr_q(pre_rot64 - GRAD_COEF * g_pre)
    final_trans = trans64 - GRAD_COEF * g_trans
    return final_rot.astype(np.float32), final_trans.astype(np.float32)
